# revision 46
# baseline (speedup 1.0000x reference)
"""Betti-matching-loss preprocessing kernel for 8 TRN2 NeuronCores.

Reference computation (per full input of shape (B=4, C=1, D=128, H=256, W=256)):
    pred_super   = 1 - maxpool3d_2x(sigmoid(input))   -> sigmoid is monotone, so
                 = sigmoid(-maxpool3d_2x(input))
    target_super = 1 - (maxpool3d_2x(target) > 0.5)   = (maxpool3d_2x(target) <= 0.5)
    out = stack([pred_super, target_super])           # (2, B, C, 64, 128, 128)

Sharding: pure data parallel. 8 shards = 4 batch samples x 2 D-halves of 64
planes each (the D split at an even index never crosses a pool window).

The kernel is pure HBM-bandwidth bound (per-core SDMA ceiling measured at
~25 B/ns/engine x 16 engines), so the wins are in moving fewer bytes:
  * the input is sent as fp16 (max-pooling commutes with any monotone
    re-encoding; sigmoid sees |err| <= 2^-11 relative, 75x inside the 2e-2
    gate) -> 8.39 MB/core;
  * binarize-then-pool commutes: maxpool(t) > 0.5  ==  maxpool(t > 0.5), so
    the target is sent BIT-PACKED (np.packbits of t > 0.5) and pooled with
    exact bitwise ORs: D/H pairs as tensor ORs, W pairs as (y | y >> 1),
    inverted+masked with one (xor 0xFF, and 0x55) op -> 0.52 MB/core;
  * pred returns as fp16 (host upcasts), target as inverted masked bits
    (host unpackbits) -> 1.18 MB/core of stores.
All DRAM params hold packed data declared as f32 words (the fast SDMA
element type); compute bitcasts SBUF tiles to fp16 / u8.

DMA geometry (microbenchmarked on this part):
  * descriptor pushes must span all 128 partitions (two complementary
    64-partition pushes run at ~0.55x the single-push rate);
  * 4-KB descriptors are the sweet spot (net 373 GB/s/core vs 306 at 1 KB);
    8-KB descriptors fall off a cliff (~0.6x) -> max_dma_last_dim=1024;
so the host emits an interleaved layout: per chunk of 8 planes, partition
p = input-row-pair holds [pair, z2, hp, two, w2] (D-pool partners are the
two contiguous 4-KB halves, each 2x2 HxW window is resolvable by two more
contiguous-halves maxes), and the SBUF tile needs no on-chip reshuffle.
The h-major output layout makes each store one 128-partition push of
contiguous 1-KB lines (the host transposes the 4-MB result back).

Schedule: ALL loads ride the sync HWDGE ring, emitted before any compute
(the whole working set is SBUF-resident) so the ring streams descriptors
back-to-back; input chunk 0 is pushed first and the target-bit compute is
slotted mid-stream so the saturated vector engine starts its first input
max as early as possible and no target work trails the input pipeline;
stores ride the independent gpsimd SWDGE queue; the scalar engine only
runs the sigmoid; the vector engine does the maxes/ORs and is co-critical
with the DMA window (~25 us each, fully overlapped).  The entry all-engine EVSEM
barrier from Bass.__init__ is skipped (-5 us): every cross-engine
dependency here is carried by tile/DMA semaphores, and the const-tile
memsets retire >10 us before their first reader.  The last chunk is
split in half to shorten the drain-down tail.

Measured: ~40 us in quiet periods (39.7-41.3), up to ~48 under heavy
device contention, vs the 130.7-us session baseline (~3.2x); rel err
2.6e-4.
"""

import numpy as np

import bass_rust
import concourse.bass as bass
import concourse.mybir as mybir
import concourse.tile as tile
from concourse.bass_utils import run_bass_kernel_spmd
from concourse.vector_clock import ScopedClock

f16 = mybir.dt.float16
f32 = mybir.dt.float32


def _patched_drain_and_barrier(self, tick_clock, wait_clock):
    """Replacement for TileContext._drain_and_barrier.

    The stock version hangs every outstanding semaphore wait on one Drain
    instruction; the walrus in this environment rejects >1 sync-wait per
    non-EventSemaphore instruction ("Too many sync wait commands").  Emit
    one sequencer NOP per semaphore wait instead, then drain + barrier.
    """
    ((_, vclock),) = ScopedClock({None: tick_clock.global_clock}).items()
    ticks = list(vclock)
    for proc_idx, sem in self.sems.allocated().items():
        t = ticks[proc_idx]
        if t > 0:
            self.nc.sync.nop()._wait_ge(sem, bass_rust.tick_to_sem(t, proc_idx))
    self.nc.sync.drain()
    self.nc.all_engine_barrier(sem_only=True)
    popped = self.nc._tile_sem_poison_stack.pop()
    assert popped is self._sem_poison
    self.nc.clear_and_free_semaphores(list(self.sems.allocated().values()))


tile.TileContext._drain_and_barrier = _patched_drain_and_barrier


def _split_excess_waits(nc: bass.Bass) -> None:
    """Walrus in this env caps sync-waits at 1 per instruction (2 for
    EventSemaphore).  Move excess waits onto same-engine NoOps inserted
    immediately before the offending instruction."""
    for f in nc.m.functions:
        for bb in f.blocks:
            insts = bb.instructions
            out = []
            changed = False
            for inst in insts:
                si = inst.sync_info
                cap = 2 if type(inst).__name__ == "InstEventSemaphore" else 1
                if si is not None and len(si.on_wait) > cap:
                    w = list(si.on_wait)
                    for k, extra in enumerate(w[cap:]):
                        nop = mybir.InstNoOp(
                            name=f"{inst.name}-xw{k}",
                            engine=inst.engine,
                            sync_info=mybir.SyncInfo(
                                on_wait=[extra], on_update=[]
                            ),
                            bass_nofuse=True,
                        )
                        nc.register_instruction(nop, overwrite=True)
                        out.append(nop)
                    inst.sync_info = mybir.SyncInfo(
                        on_wait=w[:cap], on_update=si.on_update
                    )
                    changed = True
                out.append(inst)
            if changed:
                bb.instructions = out

B, C, D, H, W = 4, 1, 128, 256, 256
NCORES = 8
D_SH = D // 2      # 64 input planes per core
DZ = D_SH // 2     # 32 output planes per core
HO, WO = H // 2, W // 2
WOP = WO // 2      # packed f32 words per output row
PPT = 8            # input planes per chunk (1 MB fp16 loads)
NT = D_SH // PPT   # full chunks per tensor
# (chunk index, z-offset within chunk in units of plane-pairs, zq pairs)
CHUNKS = [(ci, 0, PPT // 2) for ci in range(NT - 1)]
CHUNKS += [(NT - 1, 0, PPT // 4), (NT - 1, PPT // 4, PPT // 4)]

u8 = mybir.dt.uint8


def build_nc(store_ring: str = "gpsimd") -> bass.Bass:
    # Skip the ~3.5-us entry all-engine EVSEM barrier that Bass.__init__
    # emits after its const-tile memsets: the first consumer of those tiles
    # (an ACTIVATE at ~13 us) is ordered >10 us behind the sub-us memsets
    # by the tile semaphore chain, and every cross-engine dependency in
    # this kernel is carried by DMA/tile semaphores anyway.
    _orig_aeb = bass.Bass.all_engine_barrier
    bass.Bass.all_engine_barrier = lambda self, **kw: None
    try:
        nc = bass.Bass()
    finally:
        bass.Bass.all_engine_barrier = _orig_aeb
    # params are f32 views of host-interleaved packed fp16 (see _pack);
    # the target is ONE bit-packed tensor (see _pack_bits)
    inp = nc.declare_dram_parameter("input", [NT, 128, 2048], f32, isOutput=False)
    tgt = nc.declare_dram_parameter("target", [128, 1024], f32, isOutput=False)
    # h-major outputs so stores are contiguous per partition
    outp = nc.declare_dram_parameter("outp", [HO, DZ, WOP], f32, isOutput=True)
    outt = nc.declare_dram_parameter("outt", [128, 256], f32, isOutput=True)

    store_eng = getattr(nc, store_ring)

    with tile.TileContext(nc) as tc:
        with (
            tc.tile_pool(name="load", bufs=len(CHUNKS)) as load_pool,
            tc.tile_pool(name="tbits", bufs=1) as tbits_pool,
            tc.tile_pool(name="lvl1", bufs=3) as pool1,
            tc.tile_pool(name="lvl2", bufs=3) as pool2,
            tc.tile_pool(name="lvl3", bufs=3) as pool3,
            tc.tile_pool(name="post", bufs=8) as pool4,
        ):
            # ---- all loads first: with the whole working set resident no
            #      push ever waits on a slot, so the sync ring sequencer
            #      streams descriptors back-to-back ----
            tt = tbits_pool.tile([128, 1024], f32, tag="tbits")
            tiles = {}
            for ck in CHUNKS:
                ci, zo, zq = ck
                for which, src in ((0, inp),):
                    t = load_pool.tile([128, zq * 512], f32, tag="load")
                    tiles[ck, which] = t
                    # layout per partition is [pair(2), z2(4), 1 KB]: a
                    # zq-pair subchunk selects a contiguous z2 range, so
                    # each pair block stays one contiguous zq-KB span ->
                    # descriptors are 4 KB (full chunk) / 2 KB (half)
                    sv = src[ci].rearrange(
                        "p (pair z2 lin2) -> p pair z2 lin2",
                        pair=2, lin2=256,
                    )[:, :, zo:zo + zq].rearrange(
                        "p pair z2 lin2 -> p pair (z2 lin2)"
                    )
                    nc.sync.dma_start(
                        t.rearrange(
                            "p (pair lin) -> p pair lin", lin=zq * 256
                        ),
                        sv,
                        max_dma_last_dim=1024,  # 4-KB descriptors
                    )
                    if ck == CHUNKS[0]:
                        # target bits: one 512-KB push of 4-KB descriptors,
                        # AFTER chunk 0 so the DVE's first input max starts
                        # as early as possible (DVE is saturated; its end
                        # time is start + total work)
                        nc.sync.dma_start(
                            tt.rearrange(
                                "p (pair lin) -> p pair lin", lin=512
                            ),
                            tgt.rearrange(
                                "p (pair lin) -> p pair lin", lin=512
                            ),
                            max_dma_last_dim=1024,
                        )


            for ck in CHUNKS:
                ci, zo, zq = ck
                for which in (0,):
                    t = tiles[ck, which]

                    # host layout per partition: [pair, z2, hp, two, w2],
                    # so every pool level is a contiguous-run tensor_max

                    # ---- level 1: pool D (pair halves of the tile) ----
                    u = pool1.tile([128, zq * 512], f16, tag="u")
                    tb = t[:].bitcast(f16)
                    half = zq * 512
                    nc.vector.tensor_max(
                        u[:], tb[:, 0:half], tb[:, half:2 * half]
                    )

                    # ---- level 2: pool H (hp halves per plane) ----
                    v = pool2.tile([128, zq * 256], f16, tag="v")
                    uv = u.rearrange(
                        "p (z2 hp tw) -> p z2 hp tw", z2=zq, hp=2
                    )
                    nc.vector.tensor_max(
                        v.rearrange("p (z2 tw) -> p z2 tw", z2=zq),
                        uv[:, :, 0, :],
                        uv[:, :, 1, :],
                    )

                    # ---- level 3: pool W (two halves per row-group) ----
                    o = pool3.tile([128, zq * 128], f16, tag="o")
                    vv = v.rearrange(
                        "p (z2 two w2) -> p z2 two w2", z2=zq, two=2
                    )
                    nc.vector.tensor_max(
                        o.rearrange("p (z2 w2) -> p z2 w2", w2=128),
                        vv[:, :, 0, :],
                        vv[:, :, 1, :],
                    )

                    # ---- pointwise on ACT (pooled target bit b is exactly
                    #      0/1, so 1-b is a linear Identity activation) ----
                    g = pool4.tile([128, zq * 64], f32, tag="g")
                    nc.scalar.activation(
                        g[:].bitcast(f16), o[:],
                        mybir.ActivationFunctionType.Sigmoid,
                        bias=0.0, scale=-1.0,
                    )

                    # ---- store: one 128-partition push of 1-KB lines ----
                    z0 = ci * (PPT // 2) + zo
                    dst = outp[:, z0:z0 + zq, :].rearrange(
                        "p z2 wp -> p (z2 wp)"
                    )
                    store_eng.dma_start(dst, g[:])
            # ---- target: bitwise OR pooling on packed bits (exact) ----
            # per-partition layout [pair, zp(32), hp, 32B]
            ttb = tt[:].bitcast(u8)                       # [128, 4096]
            ta = pool1.tile([128, 2048], u8, tag="ta")
            nc.vector.tensor_tensor(                      # pool D
                ta[:], ttb[:, 0:2048], ttb[:, 2048:4096],
                mybir.AluOpType.bitwise_or,
            )
            tav = ta.rearrange("p (z hp l) -> p z hp l", hp=2, l=32)
            tb_ = pool2.tile([128, 1024], u8, tag="tb")
            nc.vector.tensor_tensor(                      # pool H
                tb_.rearrange("p (z l) -> p z l", l=32),
                tav[:, :, 0, :], tav[:, :, 1, :],
                mybir.AluOpType.bitwise_or,
            )
            ts_ = pool3.tile([128, 1024], u8, tag="ts")
            nc.vector.tensor_scalar(                      # shift for W pairs
                ts_[:], tb_[:], 1, None,
                mybir.AluOpType.logical_shift_right,
            )
            tz = pool3.tile([128, 1024], u8, tag="tz")
            nc.vector.tensor_tensor(                      # pool W
                tz[:], tb_[:], ts_[:], mybir.AluOpType.bitwise_or,
            )
            tg = pool4.tile([128, 256], f32, tag="tg")
            nc.vector.tensor_scalar(                      # invert + mask
                tg[:].bitcast(u8), tz[:], 0xFF, 0x55,
                mybir.AluOpType.bitwise_xor, mybir.AluOpType.bitwise_and,
            )
            store_eng.dma_start(outt[:], tg[:])
    _split_excess_waits(nc)
    return nc


_NC_CACHE: dict = {}


def _pack(a16: np.ndarray) -> np.ndarray:
    """fp16 [64, 256, 256] -> interleaved f32 [NT, 128, 2048]: index
    [ci, p, pair, z2, hp, two, w2] = a16[ci*8 + 2*z2 + pair, 2p + hp,
    2*w2 + two], so D/H/W pool partners are contiguous halves on-chip."""
    af = a16.reshape(NT, PPT // 2, 2, 128, 2, WO, 2)
    arr = np.ascontiguousarray(af.transpose(0, 3, 2, 1, 4, 6, 5))
    return arr.reshape(NT, 128, -1).view(np.float32)


def _pack_bits(tbin: np.ndarray) -> np.ndarray:
    """bool [64, 256, 256] -> bit-packed f32 [128, 1024]: partition p gets
    [pair, zp(32), hp, 32B] where plane d = 2*zp + pair, row h = 2p + hp,
    and bit k of byte j is column w = 8*j + k."""
    bits = np.packbits(tbin, axis=-1, bitorder="little")     # [64, 256, 32]
    tb = bits.reshape(32, 2, 128, 2, 32)                     # [zp,pair,p,hp,l]
    arr = np.ascontiguousarray(tb.transpose(2, 1, 0, 3, 4))  # [p,pair,zp,hp,l]
    return arr.reshape(128, -1).view(np.float32)


def make_in_maps(input: np.ndarray, target: np.ndarray) -> list:
    """Host-side prep: shard batch x D-half, downcast input to fp16 in the
    interleaved packed-f32 layout (the fast DMA geometry), and send the
    target as exact bit-packed {0,1} (binarize commutes with max)."""
    in_maps = []
    for i in range(NCORES):
        b, half = divmod(i, 2)
        sl = slice(half * D_SH, (half + 1) * D_SH)
        in_maps.append({
            "input": _pack(np.asarray(input[b, 0, sl], dtype=np.float16)),
            "target": _pack_bits(target[b, 0, sl] > 0.5),
        })
    return in_maps


def kernel(input: np.ndarray, target: np.ndarray) -> np.ndarray:
    input = np.asarray(input, dtype=np.float32)
    target = np.asarray(target, dtype=np.float32)
    assert input.shape == (B, C, D, H, W), input.shape

    if "nc" not in _NC_CACHE:
        _NC_CACHE["nc"] = build_nc()
    nc = _NC_CACHE["nc"]

    in_maps = make_in_maps(input, target)
    res = run_bass_kernel_spmd(nc, in_maps, core_ids=list(range(NCORES))).results

    full = np.empty((2, B, C, D // 2, HO, WO), dtype=np.float32)
    for i in range(NCORES):
        b, half = divmod(i, 2)
        zsl = slice(half * DZ, (half + 1) * DZ)
        # pred: [HO, DZ, WOP] packed fp16 -> z-major f32
        rp = res[i]["outp"].view(np.float16).astype(np.float32)
        full[0, b, 0, zsl] = rp.transpose(1, 0, 2)
        # target: [128, 32 z, 32 B] inverted bits at even positions
        rt = res[i]["outt"].view(np.uint8).reshape(HO, DZ, 32)
        ub = np.unpackbits(rt, axis=-1, bitorder="little")[:, :, 0::2]
        full[1, b, 0, zsl] = ub.transpose(1, 0, 2).astype(np.float32)
    return full


# revision 47
# speedup vs baseline: 1.0300x; 1.0300x over previous
"""Betti-matching-loss preprocessing kernel for 8 TRN2 NeuronCores.

Reference computation (per full input of shape (B=4, C=1, D=128, H=256, W=256)):
    pred_super   = 1 - maxpool3d_2x(sigmoid(input))   -> sigmoid is monotone, so
                 = sigmoid(-maxpool3d_2x(input))
    target_super = 1 - (maxpool3d_2x(target) > 0.5)   = (maxpool3d_2x(target) <= 0.5)
    out = stack([pred_super, target_super])           # (2, B, C, 64, 128, 128)

Sharding: pure data parallel. 8 shards = 4 batch samples x 2 D-halves of 64
planes each (the D split at an even index never crosses a pool window).

The kernel is pure HBM-bandwidth bound (per-core SDMA ceiling measured at
~25 B/ns/engine x 16 engines), so the wins are in moving fewer bytes:
  * the input is sent as fp16 (max-pooling commutes with any monotone
    re-encoding; sigmoid sees |err| <= 2^-11 relative, 75x inside the 2e-2
    gate) -> 8.39 MB/core;
  * binarize-then-pool commutes: maxpool(t) > 0.5  ==  maxpool(t > 0.5), so
    the target is sent BIT-PACKED (np.packbits of t > 0.5) and pooled with
    exact bitwise ORs: D/H pairs as tensor ORs, W pairs as (y | y >> 1),
    inverted+masked with one (xor 0xFF, and 0x55) op -> 0.52 MB/core;
  * pred returns as fp16 (host upcasts), target as inverted masked bits
    (host unpackbits) -> 1.18 MB/core of stores.
All DRAM params hold packed data declared as f32 words (the fast SDMA
element type); compute bitcasts SBUF tiles to fp16 / u8.

DMA geometry (microbenchmarked on this part):
  * descriptor pushes must span all 128 partitions (two complementary
    64-partition pushes run at ~0.55x the single-push rate);
  * 4-KB descriptors are the sweet spot (net 373 GB/s/core vs 306 at 1 KB);
    8-KB descriptors fall off a cliff (~0.6x) -> max_dma_last_dim=1024;
so the host emits an interleaved layout: per chunk of 8 planes, partition
p = input-row-pair holds [pair, z2, hp, two, w2] (D-pool partners are the
two contiguous 4-KB halves, each 2x2 HxW window is resolvable by two more
contiguous-halves maxes), and the SBUF tile needs no on-chip reshuffle.
The h-major output layout makes each store one 128-partition push of
contiguous 1-KB lines (the host transposes the 4-MB result back).

Schedule: ALL loads ride the sync HWDGE ring, emitted before any compute
(the whole working set is SBUF-resident) so the ring streams descriptors
back-to-back; input chunk 0 is pushed first and the target-bit compute is
slotted mid-stream so the saturated vector engine starts its first input
max as early as possible and no target work trails the input pipeline;
stores ride the independent gpsimd SWDGE queue; the scalar engine only
runs the sigmoid; the vector engine does the maxes/ORs and is co-critical
with the DMA window (~25 us each, fully overlapped).  The entry all-engine EVSEM
barrier from Bass.__init__ is skipped (-5 us): every cross-engine
dependency here is carried by tile/DMA semaphores, and the const-tile
memsets retire >10 us before their first reader.  The last chunk is
split in half to shorten the drain-down tail.

Measured: ~40 us in quiet periods (39.7-41.3), up to ~48 under heavy
device contention, vs the 130.7-us session baseline (~3.2x); rel err
2.6e-4.
"""

import numpy as np

import bass_rust
import concourse.bass as bass
import concourse.mybir as mybir
import concourse.tile as tile
from concourse.bass_utils import run_bass_kernel_spmd
from concourse.vector_clock import ScopedClock

f16 = mybir.dt.float16
f32 = mybir.dt.float32


def _patched_drain_and_barrier(self, tick_clock, wait_clock):
    """Replacement for TileContext._drain_and_barrier.

    The stock version hangs every outstanding semaphore wait on one Drain
    instruction; the walrus in this environment rejects >1 sync-wait per
    non-EventSemaphore instruction ("Too many sync wait commands").  Emit
    one sequencer NOP per semaphore wait instead, then drain + barrier.
    """
    ((_, vclock),) = ScopedClock({None: tick_clock.global_clock}).items()
    ticks = list(vclock)
    for proc_idx, sem in self.sems.allocated().items():
        t = ticks[proc_idx]
        if t > 0:
            self.nc.sync.nop()._wait_ge(sem, bass_rust.tick_to_sem(t, proc_idx))
    self.nc.sync.drain()
    self.nc.all_engine_barrier(sem_only=True)
    popped = self.nc._tile_sem_poison_stack.pop()
    assert popped is self._sem_poison
    self.nc.clear_and_free_semaphores(list(self.sems.allocated().values()))


tile.TileContext._drain_and_barrier = _patched_drain_and_barrier


def _split_excess_waits(nc: bass.Bass) -> None:
    """Walrus in this env caps sync-waits at 1 per instruction (2 for
    EventSemaphore).  Move excess waits onto same-engine NoOps inserted
    immediately before the offending instruction."""
    for f in nc.m.functions:
        for bb in f.blocks:
            insts = bb.instructions
            out = []
            changed = False
            for inst in insts:
                si = inst.sync_info
                cap = 2 if type(inst).__name__ == "InstEventSemaphore" else 1
                if si is not None and len(si.on_wait) > cap:
                    w = list(si.on_wait)
                    for k, extra in enumerate(w[cap:]):
                        nop = mybir.InstNoOp(
                            name=f"{inst.name}-xw{k}",
                            engine=inst.engine,
                            sync_info=mybir.SyncInfo(
                                on_wait=[extra], on_update=[]
                            ),
                            bass_nofuse=True,
                        )
                        nc.register_instruction(nop, overwrite=True)
                        out.append(nop)
                    inst.sync_info = mybir.SyncInfo(
                        on_wait=w[:cap], on_update=si.on_update
                    )
                    changed = True
                out.append(inst)
            if changed:
                bb.instructions = out

B, C, D, H, W = 4, 1, 128, 256, 256
NCORES = 8
D_SH = D // 2      # 64 input planes per core
DZ = D_SH // 2     # 32 output planes per core
HO, WO = H // 2, W // 2
WOP = WO // 2      # packed f32 words per output row
PPT = 8            # input planes per chunk (1 MB fp16 loads)
NT = D_SH // PPT   # full chunks per tensor
# (chunk index, z-offset within chunk in units of plane-pairs, zq pairs)
CHUNKS = [(ci, 0, PPT // 2) for ci in range(NT - 1)]
CHUNKS += [(NT - 1, 0, PPT // 4), (NT - 1, PPT // 4, PPT // 4)]

u8 = mybir.dt.uint8
i8 = mybir.dt.int8


def build_nc(store_ring: str = "gpsimd") -> bass.Bass:
    # Skip the ~3.5-us entry all-engine EVSEM barrier that Bass.__init__
    # emits after its const-tile memsets: the first consumer of those tiles
    # (an ACTIVATE at ~13 us) is ordered >10 us behind the sub-us memsets
    # by the tile semaphore chain, and every cross-engine dependency in
    # this kernel is carried by DMA/tile semaphores anyway.
    _orig_aeb = bass.Bass.all_engine_barrier
    bass.Bass.all_engine_barrier = lambda self, **kw: None
    try:
        nc = bass.Bass()
    finally:
        bass.Bass.all_engine_barrier = _orig_aeb
    # params are f32 views of host-interleaved packed fp16 (see _pack);
    # the target is ONE bit-packed tensor (see _pack_bits)
    inp = nc.declare_dram_parameter("input", [NT, 128, 4096], i8, isOutput=False)
    tgt = nc.declare_dram_parameter("target", [128, 1024], f32, isOutput=False)
    # h-major outputs so stores are contiguous per partition
    outp = nc.declare_dram_parameter("outp", [HO, DZ, WOP], f32, isOutput=True)
    outt = nc.declare_dram_parameter("outt", [128, 256], f32, isOutput=True)

    store_eng = getattr(nc, store_ring)

    with tile.TileContext(nc) as tc:
        with (
            tc.tile_pool(name="load", bufs=len(CHUNKS)) as load_pool,
            tc.tile_pool(name="tbits", bufs=1) as tbits_pool,
            tc.tile_pool(name="lvl1", bufs=3) as pool1,
            tc.tile_pool(name="lvl2", bufs=3) as pool2,
            tc.tile_pool(name="lvl3", bufs=3) as pool3,
            tc.tile_pool(name="post", bufs=8) as pool4,
        ):
            # ---- all loads first: with the whole working set resident no
            #      push ever waits on a slot, so the sync ring sequencer
            #      streams descriptors back-to-back ----
            tt = tbits_pool.tile([128, 1024], f32, tag="tbits")
            tiles = {}
            for ck in CHUNKS:
                ci, zo, zq = ck
                for which, src in ((0, inp),):
                    t = load_pool.tile([128, zq * 1024], f16, tag="load")
                    tiles[ck, which] = t
                    # layout per partition is [pair(2), z2(4), 1 KB]: a
                    # zq-pair subchunk selects a contiguous z2 range, so
                    # each pair block stays one contiguous zq-KB span ->
                    # descriptors are 4 KB (full chunk) / 2 KB (half)
                    sv = src[ci].rearrange(
                        "p (pair z2 lin2) -> p pair z2 lin2",
                        pair=2, lin2=512,
                    )[:, :, zo:zo + zq].rearrange(
                        "p pair z2 lin2 -> p pair (z2 lin2)"
                    )
                    nc.gpsimd.dma_start(
                        t.rearrange(
                            "p (pair lin) -> p pair lin", lin=zq * 512
                        ),
                        sv,
                        max_dma_last_dim=2048,
                    )
                    if ck == CHUNKS[0]:
                        # target bits: one 512-KB push of 4-KB descriptors,
                        # AFTER chunk 0 so the DVE's first input max starts
                        # as early as possible (DVE is saturated; its end
                        # time is start + total work)
                        nc.sync.dma_start(
                            tt.rearrange(
                                "p (pair lin) -> p pair lin", lin=512
                            ),
                            tgt.rearrange(
                                "p (pair lin) -> p pair lin", lin=512
                            ),
                            max_dma_last_dim=1024,
                        )


            for ck in CHUNKS:
                ci, zo, zq = ck
                for which in (0,):
                    t = tiles[ck, which]

                    # host layout per partition: [pair, z2, hp, two, w2],
                    # so every pool level is a contiguous-run tensor_max

                    # ---- level 1: pool D (pair halves of the tile) ----
                    u = pool1.tile([128, zq * 512], f16, tag="u")
                    tb = t[:]
                    half = zq * 512
                    nc.vector.tensor_max(
                        u[:], tb[:, 0:half], tb[:, half:2 * half]
                    )

                    # ---- level 2: pool H (hp halves per plane) ----
                    v = pool2.tile([128, zq * 256], f16, tag="v")
                    uv = u.rearrange(
                        "p (z2 hp tw) -> p z2 hp tw", z2=zq, hp=2
                    )
                    nc.vector.tensor_max(
                        v.rearrange("p (z2 tw) -> p z2 tw", z2=zq),
                        uv[:, :, 0, :],
                        uv[:, :, 1, :],
                    )

                    # ---- level 3: pool W (two halves per row-group) ----
                    o = pool3.tile([128, zq * 128], f16, tag="o")
                    vv = v.rearrange(
                        "p (z2 two w2) -> p z2 two w2", z2=zq, two=2
                    )
                    nc.vector.tensor_max(
                        o.rearrange("p (z2 w2) -> p z2 w2", w2=128),
                        vv[:, :, 0, :],
                        vv[:, :, 1, :],
                    )

                    # ---- pointwise on ACT (pooled target bit b is exactly
                    #      0/1, so 1-b is a linear Identity activation) ----
                    g = pool4.tile([128, zq * 64], f32, tag="g")
                    nc.scalar.activation(
                        g[:].bitcast(f16), o[:],
                        mybir.ActivationFunctionType.Sigmoid,
                        bias=0.0, scale=-1.0 / 25.5,
                    )

                    # ---- store: one 128-partition push of 1-KB lines ----
                    z0 = ci * (PPT // 2) + zo
                    dst = outp[:, z0:z0 + zq, :].rearrange(
                        "p z2 wp -> p (z2 wp)"
                    )
                    store_eng.dma_start(dst, g[:])
            # ---- target: bitwise OR pooling on packed bits (exact) ----
            # per-partition layout [pair, zp(32), hp, 32B]
            ttb = tt[:].bitcast(u8)                       # [128, 4096]
            ta = pool1.tile([128, 2048], u8, tag="ta")
            nc.vector.tensor_tensor(                      # pool D
                ta[:], ttb[:, 0:2048], ttb[:, 2048:4096],
                mybir.AluOpType.bitwise_or,
            )
            tav = ta.rearrange("p (z hp l) -> p z hp l", hp=2, l=32)
            tb_ = pool2.tile([128, 1024], u8, tag="tb")
            nc.vector.tensor_tensor(                      # pool H
                tb_.rearrange("p (z l) -> p z l", l=32),
                tav[:, :, 0, :], tav[:, :, 1, :],
                mybir.AluOpType.bitwise_or,
            )
            ts_ = pool3.tile([128, 1024], u8, tag="ts")
            nc.vector.tensor_scalar(                      # shift for W pairs
                ts_[:], tb_[:], 1, None,
                mybir.AluOpType.logical_shift_right,
            )
            tz = pool3.tile([128, 1024], u8, tag="tz")
            nc.vector.tensor_tensor(                      # pool W
                tz[:], tb_[:], ts_[:], mybir.AluOpType.bitwise_or,
            )
            tg = pool4.tile([128, 256], f32, tag="tg")
            nc.vector.tensor_scalar(                      # invert + mask
                tg[:].bitcast(u8), tz[:], 0xFF, 0x55,
                mybir.AluOpType.bitwise_xor, mybir.AluOpType.bitwise_and,
            )
            store_eng.dma_start(outt[:], tg[:])
    _split_excess_waits(nc)
    return nc


_NC_CACHE: dict = {}


def _pack(x: np.ndarray) -> np.ndarray:
    """f32 [64, 256, 256] -> monotone int8 codes q = round(25.5*x) (clamped
    at +-5.02 where sigmoid saturates) in the interleaved layout
    [ci, p, pair, z2, hp, two, w2]; the SWDGE cast-DMA widens them to fp16
    on chip and the sigmoid's scale decodes the quantization."""
    q8 = np.clip(np.rint(x * np.float32(25.5)), -128, 127).astype(np.int8)
    af = q8.reshape(NT, PPT // 2, 2, 128, 2, WO, 2)
    arr = np.ascontiguousarray(af.transpose(0, 3, 2, 1, 4, 6, 5))
    return arr.reshape(NT, 128, -1)


def _pack_bits(tbin: np.ndarray) -> np.ndarray:
    """bool [64, 256, 256] -> bit-packed f32 [128, 1024]: partition p gets
    [pair, zp(32), hp, 32B] where plane d = 2*zp + pair, row h = 2p + hp,
    and bit k of byte j is column w = 8*j + k."""
    bits = np.packbits(tbin, axis=-1, bitorder="little")     # [64, 256, 32]
    tb = bits.reshape(32, 2, 128, 2, 32)                     # [zp,pair,p,hp,l]
    arr = np.ascontiguousarray(tb.transpose(2, 1, 0, 3, 4))  # [p,pair,zp,hp,l]
    return arr.reshape(128, -1).view(np.float32)


def make_in_maps(input: np.ndarray, target: np.ndarray) -> list:
    """Host-side prep: shard batch x D-half, downcast input to fp16 in the
    interleaved packed-f32 layout (the fast DMA geometry), and send the
    target as exact bit-packed {0,1} (binarize commutes with max)."""
    in_maps = []
    for i in range(NCORES):
        b, half = divmod(i, 2)
        sl = slice(half * D_SH, (half + 1) * D_SH)
        in_maps.append({
            "input": _pack(input[b, 0, sl]),
            "target": _pack_bits(target[b, 0, sl] > 0.5),
        })
    return in_maps


def kernel(input: np.ndarray, target: np.ndarray) -> np.ndarray:
    input = np.asarray(input, dtype=np.float32)
    target = np.asarray(target, dtype=np.float32)
    assert input.shape == (B, C, D, H, W), input.shape

    if "nc" not in _NC_CACHE:
        _NC_CACHE["nc"] = build_nc()
    nc = _NC_CACHE["nc"]

    in_maps = make_in_maps(input, target)
    res = run_bass_kernel_spmd(nc, in_maps, core_ids=list(range(NCORES))).results

    full = np.empty((2, B, C, D // 2, HO, WO), dtype=np.float32)
    for i in range(NCORES):
        b, half = divmod(i, 2)
        zsl = slice(half * DZ, (half + 1) * DZ)
        # pred: [HO, DZ, WOP] packed fp16 -> z-major f32
        rp = res[i]["outp"].view(np.float16).astype(np.float32)
        full[0, b, 0, zsl] = rp.transpose(1, 0, 2)
        # target: [128, 32 z, 32 B] inverted bits at even positions
        rt = res[i]["outt"].view(np.uint8).reshape(HO, DZ, 32)
        ub = np.unpackbits(rt, axis=-1, bitorder="little")[:, :, 0::2]
        full[1, b, 0, zsl] = ub.transpose(1, 0, 2).astype(np.float32)
    return full
